# revision 6
# baseline (speedup 1.0000x reference)
"""Trainium2 Bass kernel for nn_BlockShuffleLayer (butterfly block-diag MLP).

Math (reference):
  out1[b, k, q] = sum_p x[b, k*256+p] * w1[k, q, p]          (k=16 blocks, p=q=256)
  shuffle: kq index (k*256+q) viewed as (r, l), r=kq//16, l=kq%16
  out2[b, s, l] = sum_r out1s[b, l, r] * w2[l, s, r]          (l=16 blocks, r=256, s=1024)
  out[b, s*16+l] = out2[b, s, l]

Strategy: data-parallel over the 4096-token batch across 8 cores (512 tokens
each), weights replicated.  Per core, tokens live on PSUM/SBUF partitions:

  phase A (per 128-token chunk):
    - load x rows, transpose 128x128 tiles on PE (contract dim must be on
      partitions)
    - stage-1 matmuls with w1 pre-permuted on host so its output columns are
      ordered (l, j) with k innermost; the butterfly shuffle then reduces to
      transposing *contiguous* 128-column slices
  phase B (stream w2 in s-quarters, double-buffered):
    - stage-2 matmuls z^T @ w2 -> psum[b, s]
    - scatter-copy psum into the interleaved output layout (stride-16 SBUF
      writes are free for DVE/ACT), then one big contiguous DMA per chunk.
"""

import numpy as np

import concourse.bacc as bacc
import concourse.bass as bass
import concourse.mybir as mybir
import concourse.tile as tile
from concourse import bass_utils
from concourse.masks import make_identity

FP32 = mybir.dt.float32

K, Q, P = 16, 256, 256
L, S, R = 16, 1024, 256
N_IN = K * P          # 4096
N_OUT = S * L         # 16384
BATCH = 4096
NCORES = 8
NSQ = 4               # stream w2 in s-quarters
SQW = S // NSQ        # 256


def build_kernel(n_tokens: int = BATCH // NCORES, reps: int = 1) -> bass.Bass:
    nbc = n_tokens // 128
    nc = bacc.Bacc("TRN2", target_bir_lowering=False, debug=False,
                   num_devices=NCORES)

    xs = nc.dram_tensor("xs", [n_tokens, N_IN], FP32, kind="ExternalInput")
    # host-prepared layouts (see _prep_weights):
    #   w1t[p, k, pc, q'] = w1[k, q_of(q'), pc*128+p],  q_of(q') = (q'%16)*16 + q'//16
    #   w2t[sq, u, l, h, s'] = w2[l, sq*SQW+s', (u%16)*16 + 8h + u//16]
    w1t = nc.dram_tensor("w1t", [128, K, 2, Q], FP32, kind="ExternalInput")
    w2t = nc.dram_tensor("w2t", [NSQ, 128, L, 2, SQW], FP32, kind="ExternalInput")
    out = nc.dram_tensor("out", [n_tokens, N_OUT], FP32, kind="ExternalOutput")

    with tile.TileContext(nc) as tc:
        with tc.tile_pool(name="const", bufs=1) as cpool:
            ident = cpool.tile([128, 128], FP32)
            make_identity(nc, ident[:])
            # z[r%128, l, rc, bc, b]: shuffled stage-1 output, feature-major
            z_sb = cpool.tile([128, L, 2, nbc, 128], FP32)

            def phase_a():
                with tc.tile_pool(name="pa", bufs=2) as pa, \
                     tc.tile_pool(name="paw", bufs=1) as paw, \
                     tc.tile_pool(name="pap", bufs=2, space="PSUM") as pap:
                    w1_sb = paw.tile([128, K, 2, Q], FP32, name="w1_sb")
                    nc.sync.dma_start(w1_sb[:], w1t[:])
                    for bc in range(nbc):
                        # out1[b, q', k]: k innermost so the shuffle gather for
                        # (l, r-half) is a CONTIGUOUS 128-column slice
                        # (matmul APs allow only one free dim)
                        out1 = pa.tile([128, Q, K], FP32, tag="out1", name="out1")
                        for k in range(K):
                            xk = pa.tile([128, P], FP32, tag="xk", name="xk")
                            nc.sync.dma_start(
                                xk[:],
                                xs[bc * 128:(bc + 1) * 128, k * P:(k + 1) * P])
                            xT = pa.tile([128, 2, 128], FP32, tag="xT", name="xT")
                            for pc in range(2):
                                pt = pap.tile([128, 128], FP32, tag="pt", name="pt")
                                nc.tensor.transpose(
                                    pt[:], xk[:, pc * 128:(pc + 1) * 128], ident[:])
                                nc.vector.tensor_copy(xT[:, pc, :], pt[:])
                            ps1 = pap.tile([128, Q], FP32, tag="ps1", name="ps1")
                            for pc in range(2):
                                nc.tensor.matmul(ps1[:], xT[:, pc, :],
                                                 w1_sb[:, k, pc, :],
                                                 start=(pc == 0), stop=(pc == 1))
                            nc.scalar.copy(out1[:, :, k], ps1[:])
                        # butterfly shuffle: columns [l*256+128h, +128) of the
                        # flat (q', k) view hold blocks k=0..15 x j=8h..8h+8;
                        # transpose them: z row u <-> r = (u%16)*16 + 8h + u//16
                        # (w2 is r-permuted on host to match)
                        out1f = out1[:].rearrange("b q k -> b (q k)")
                        for l in range(L):
                            for h in range(2):
                                pz = pap.tile([128, 128], FP32, tag="pz", name="pz")
                                nc.tensor.transpose(
                                    pz[:],
                                    out1f[:, l * 256 + 128 * h:
                                          l * 256 + 128 * h + 128],
                                    ident[:])
                                nc.vector.tensor_copy(z_sb[:, l, h, bc, :], pz[:])

            def phase_b():
                with tc.tile_pool(name="pb", bufs=2) as pb, \
                     tc.tile_pool(name="pbp", bufs=4, space="PSUM") as pbp:
                    for sq in range(NSQ):
                        w2q = pb.tile([128, L, 2, SQW], FP32, tag="w2q", name="w2q")
                        nc.sync.dma_start(w2q[:], w2t[sq])
                        for bc in range(nbc):
                            ob = pb.tile([128, L * SQW], FP32, tag="ob", name="ob")
                            ob3 = ob[:].rearrange("p (s l) -> p s l", l=L)
                            for l in range(L):
                                ps2 = pbp.tile([128, SQW], FP32, tag="ps2",
                                               name="ps2")
                                for h in range(2):
                                    nc.tensor.matmul(ps2[:], z_sb[:, l, h, bc, :],
                                                     w2q[:, l, h, :],
                                                     start=(h == 0), stop=(h == 1))
                                if l % 2 == 0:
                                    nc.vector.tensor_copy(ob3[:, :, l], ps2[:])
                                else:
                                    nc.scalar.copy(ob3[:, :, l], ps2[:])
                            nc.sync.dma_start(
                                out[bc * 128:(bc + 1) * 128,
                                    sq * L * SQW:(sq + 1) * L * SQW],
                                ob[:])

            for _rep in range(reps):
                phase_a()
                phase_b()

    nc.compile()
    return nc


_QPERM = np.array([(v % 16) * 16 + v // 16 for v in range(Q)])
# z-chunk h row u holds r = (u%16)*16 + 8h + u//16
_RPERM = np.array([[(u % 16) * 16 + 8 * h + u // 16 for u in range(128)]
                   for h in range(2)])


def _prep_weights(w1: np.ndarray, w2: np.ndarray):
    # w1t[p, k, pc, q'] = w1[k, q_of(q'), pc*128+p]
    w1p = w1[:, _QPERM, :]                       # [k, q', P]
    w1t = np.ascontiguousarray(
        w1p.reshape(K, Q, 2, 128).transpose(3, 0, 2, 1))
    # w2t[sq, u, l, h, s'] = w2[l, sq*SQW+s', _RPERM[h, u]]
    w2r = w2[:, :, _RPERM]                       # [l, s, h, u]
    w2t = np.ascontiguousarray(
        w2r.reshape(L, NSQ, SQW, 2, 128).transpose(1, 4, 0, 3, 2))
    return w1t, w2t


_NC_CACHE: dict = {}


def kernel(x, w1, w2) -> np.ndarray:
    x = np.ascontiguousarray(np.asarray(x, dtype=np.float32))
    w1 = np.asarray(w1, dtype=np.float32)
    w2 = np.asarray(w2, dtype=np.float32)
    assert x.shape == (BATCH, N_IN) and w1.shape == (K, Q, P) \
        and w2.shape == (L, S, R)

    if "nc" not in _NC_CACHE:
        _NC_CACHE["nc"] = build_kernel(BATCH // NCORES)
    nc = _NC_CACHE["nc"]

    w1t, w2t = _prep_weights(w1, w2)
    shard = BATCH // NCORES
    in_maps = [
        {"xs": x[i * shard:(i + 1) * shard], "w1t": w1t, "w2t": w2t}
        for i in range(NCORES)
    ]
    res = bass_utils.run_bass_kernel_spmd(nc, in_maps,
                                          core_ids=list(range(NCORES)))
    return np.concatenate([r["out"] for r in res.results], axis=0)


# revision 15
# speedup vs baseline: 438.1930x; 438.1930x over previous
"""Trainium2 Bass kernel for nn_BlockShuffleLayer (butterfly block-diag MLP).

Math (reference):
  out1[b, k, q] = sum_p x[b, k*256+p] * w1[k, q, p]          (k=16 blocks, p=q=256)
  shuffle: kq index (k*256+q) viewed as (r, l), r=kq//16, l=kq%16
  out2[b, s, l] = sum_r out1s[b, l, r] * w2[l, s, r]          (l=16 blocks, r=256, s=1024)
  out[b, s*16+l] = out2[b, s, l]

Strategy: data-parallel over the 4096-token batch across 8 cores (512 tokens
each), weights replicated.  Per core:

  phase A (stage 1, output feature-major):
    - x arrives host-transposed (xt[p, b]) so the contraction dim is already
      on partitions: zero on-chip transposes, and the tensor engine runs a
      pure back-to-back matmul stream (keeps the HAM clock warm).
    - stage-1 matmuls produce out1T[q'', b] in PSUM with w1 column-permuted
      on host so that one DVE copy + ONE SBUF->SBUF DMA per psum tile
      scatters 8 16-partition stripes into the butterfly-shuffled z layout.
  phase B (stage 2, tokens-major):
    - w2 streamed in s-halves (N=512 matmuls, 256 total)
    - psum[b, s] scatter-copied (stride-16 writes) into the interleaved
      output columns, then contiguous 2MB DMAs to DRAM.
"""

import numpy as np

import concourse.bacc as bacc
import concourse.bass as bass
import concourse.mybir as mybir
import concourse.tile as tile
from concourse import bass_utils

FP32 = mybir.dt.float32
# float32r: fp32 operands with single-pass (relaxed-precision) multiply --
# 4x PE throughput vs true fp32; HW-measured rel err ~1.6e-4 per 128-deep
# contraction (TF32-class).  Set to mybir.dt.float32 for exact fp32.
MMDT = mybir.dt.float32r

K, Q, P = 16, 256, 256
L, S, R = 16, 1024, 256
N_IN = K * P          # 4096
N_OUT = S * L         # 16384
BATCH = 4096
NCORES = 8
SHARD = BATCH // NCORES


def build_kernel(n_tokens: int = SHARD, reps: int = 1) -> bass.Bass:
    nbc = n_tokens // 128
    nc = bacc.Bacc("TRN2", target_bir_lowering=False, debug=False,
                   num_devices=NCORES)

    # host-prepared layouts (see _prep_weights / kernel):
    #   xt[P, b]                      = x[b, P]  (pre-transposed shard)
    #   w1t[p, k, pc, qc*128+u]       = w1[k, (u//8)*16 + qc*8 + u%8, pc*128+p]
    #   w2t[sh, r', l, rc, s']        = w2[l, sh*512+s', rc*128+r']
    xt = nc.dram_tensor("xt", [N_IN, n_tokens], FP32, kind="ExternalInput")
    w1t = nc.dram_tensor("w1t", [128, K, 2, Q], FP32, kind="ExternalInput")
    w2t = nc.dram_tensor("w2t", [2, 128, L, 2, 512], FP32, kind="ExternalInput")
    out = nc.dram_tensor("out", [n_tokens, N_OUT], FP32, kind="ExternalOutput")

    with tile.TileContext(nc) as tc:
        with tc.tile_pool(name="const", bufs=1) as cpool:
            # z[u', l, rc, b]: shuffled stage-1 output; r = rc*128 + u'
            z_sb = cpool.tile([128, L, 2, n_tokens], MMDT)
            # w2 s-half as 16 per-l tiles: the second-half reload of tile l
            # only waits for *its own* first-half readers, overlapping the
            # 8MB reload with compute instead of a bulk WAR stall
            w2h = [cpool.tile([128, 2, 512], MMDT, name=f"w2h{l}")
                   for l in range(L)]

            def phase_a():
                with tc.tile_pool(name="pa", bufs=1) as pa, \
                     tc.tile_pool(name="pap", bufs=6, space="PSUM") as pap:
                    w1_sb = pa.tile([128, K, 2, Q], MMDT, name="w1_sb")
                    nc.sync.dma_start(w1_sb[:], w1t[:].bitcast(MMDT))
                    # process k-pairs (k0, k0+8): their stripes land in the
                    # same 16 z partitions (differing only in the rc slot),
                    # so one DMA per (pair, t) scatters 4 stripes at once
                    for k0 in range(8):
                        # prefetch first w2 half spread behind stage-1 compute
                        for l in (2 * k0, 2 * k0 + 1):
                            nc.scalar.dma_start(w2h[l][:],
                                                w2t[0, :, l].bitcast(MMDT))
                        stg = pa.tile([128, 2, 2, n_tokens], MMDT, tag="stg",
                                      name="stg", bufs=2)   # [u, qc, kh, b]
                        for kh in range(2):
                            k = k0 + 8 * kh
                            xtk = pa.tile([128, 2, n_tokens], MMDT, tag="xtk",
                                          name="xtk", bufs=3)
                            nc.sync.dma_start(
                                xtk[:],
                                xt[k * P:(k + 1) * P].rearrange(
                                    "(pc p) b -> p pc b", p=128)
                                .bitcast(MMDT))
                            for qc in range(2):
                                ps1 = pap.tile([128, n_tokens], FP32,
                                               tag="ps1", name="ps1")
                                for pc in range(2):
                                    nc.tensor.matmul(
                                        ps1[:],
                                        w1_sb[:, k, pc,
                                              qc * 128:(qc + 1) * 128],
                                        xtk[:, pc, :],
                                        start=(pc == 0), stop=(pc == 1))
                                if (kh + qc) % 2 == 0:
                                    nc.vector.tensor_copy(
                                        stg[:, qc, kh, :], ps1[:])
                                else:
                                    nc.scalar.copy(stg[:, qc, kh, :], ps1[:])
                        # butterfly redistribution: psum partition u = 16t+j
                        # holds column (l = qc*8+t, j); z row u' = k0*16+j,
                        # rc = kh, so r = rc*128+u' is natural for w2.
                        for t in range(8):
                            nc.sync.dma_start(
                                z_sb[k0 * 16:k0 * 16 + 16, t:t + 9:8, :, :],
                                stg[16 * t:16 * t + 16, :, :, :])

            def phase_b():
                with tc.tile_pool(name="pb", bufs=4) as pb, \
                     tc.tile_pool(name="pbp", bufs=6, space="PSUM") as pbp:
                    for sh in range(2):
                        if sh == 1:
                            for l in range(L):
                                nc.scalar.dma_start(w2h[l][:],
                                                    w2t[1, :, l].bitcast(MMDT))
                        for bc in range(nbc):
                            obs = []
                            for qq in range(2):
                                ob = pb.tile([128, L * 256], FP32, tag="ob",
                                             name="ob")
                                obs.append(ob)
                            for l in range(L):
                                ps2 = pbp.tile([128, 512], FP32, tag="ps2",
                                               name="ps2")
                                for rc in range(2):
                                    nc.tensor.matmul(
                                        ps2[:],
                                        z_sb[:, l, rc, bc * 128:(bc + 1) * 128],
                                        w2h[l][:, rc, :],
                                        start=(rc == 0), stop=(rc == 1))
                                for qq in range(2):
                                    ob3 = obs[qq][:].rearrange(
                                        "p (s l) -> p s l", l=L)
                                    if (l + qq) % 2 == 0:
                                        nc.vector.tensor_copy(
                                            ob3[:, :, l],
                                            ps2[:, qq * 256:(qq + 1) * 256])
                                    else:
                                        nc.scalar.copy(
                                            ob3[:, :, l],
                                            ps2[:, qq * 256:(qq + 1) * 256])
                            for qq in range(2):
                                c0 = sh * 8192 + qq * 4096
                                nc.sync.dma_start(
                                    out[bc * 128:(bc + 1) * 128, c0:c0 + 4096],
                                    obs[qq][:])

            for _rep in range(reps):
                phase_a()
                phase_b()

    nc.compile()
    return nc


# stage-1 psum chunk qc, partition u = 16t+j holds output column
# q = j*16 + (qc*8 + t)
_QCOL = np.array([(u % 16) * 16 + (qc * 8) + u // 16
                  for qc in range(2) for u in range(128)])


def _prep_weights(w1: np.ndarray, w2: np.ndarray):
    # w1t[p, k, pc, q''] = w1[k, _QCOL[q''], pc*128+p]
    w1p = w1[:, _QCOL, :]                        # [k, q'', P]
    w1t = np.ascontiguousarray(
        w1p.reshape(K, Q, 2, 128).transpose(3, 0, 2, 1))
    # w2t[sh, r', l, rc, s'] = w2[l, sh*512+s', rc*128+r']
    w2t = np.ascontiguousarray(
        w2.reshape(L, 2, 512, 2, 128).transpose(1, 4, 0, 3, 2))
    return w1t, w2t


_NC_CACHE: dict = {}


def kernel(x, w1, w2) -> np.ndarray:
    x = np.asarray(x, dtype=np.float32)
    w1 = np.asarray(w1, dtype=np.float32)
    w2 = np.asarray(w2, dtype=np.float32)
    assert x.shape == (BATCH, N_IN) and w1.shape == (K, Q, P) \
        and w2.shape == (L, S, R)

    if "nc" not in _NC_CACHE:
        _NC_CACHE["nc"] = build_kernel(SHARD)
    nc = _NC_CACHE["nc"]

    w1t, w2t = _prep_weights(w1, w2)
    in_maps = [
        {"xt": np.ascontiguousarray(x[i * SHARD:(i + 1) * SHARD].T),
         "w1t": w1t, "w2t": w2t}
        for i in range(NCORES)
    ]
    res = bass_utils.run_bass_kernel_spmd(nc, in_maps,
                                          core_ids=list(range(NCORES)))
    return np.concatenate([r["out"] for r in res.results], axis=0)
